# revision 37
# baseline (speedup 1.0000x reference)
"""CTC loss (warp-ctc semantics, size_average=True) on 8 Trainium2 NeuronCores.

Strategy (data-parallel over batch, 4 samples per core):
- Z[t,b] = sum_v exp(acts[t,b,v]) streamed as [128,8000] tiles; exp + free-dim
  sum fused in one ScalarE activation (accum_out) accumulating into an SBUF
  column; one tiny DMA at the end. Host does log Z in float64.
- The alpha recursion runs in the LINEAR domain on p~ = exp(e - c_t), where
  c_t is a host-computed per-(t,sample) normalization constant chosen so the
  recursion stays centered in fp32/bf16 range with NO device rescaling (the
  constants are folded back in log-space on the host, exploiting linearity).
- Forward + backward from both ends, meeting at T/2: the CTC lattice is
  symmetric under time+state reversal (no adjacent repeats), so the backward
  suffix probabilities come from the SAME recursion run on a reversed
  emission table. Both chains are FUSED into shared 16-wide tiles, so each
  time index is ONE bf16 matmul (shift, one resident stationary) plus three
  DVE ops covering both chains:
    q = alpha[:, 8:24] + alpha[:, 0:16]   # [blank|label+blank], zeros trick
    ps = w_shift^T @ dup(label cols)      # TensorE -> PSUM, both chains
    y = q + ps ; alpha' = y * phat_t      # DVE
- Alpha layout [101, 24]: cols 0-7 permanently zero, 8-15 blank (fwd 4, bwd
  4), 16-23 label (fwd 4, bwd 4).
- Final: host combines alpha_mid/beta_mid across the cut in float64 with one
  transition application; ll = log P + sum c_t - sum log Z; loss = -mean(ll).
"""

import sys
import types

import numpy as np
import ml_dtypes

# ---- shim: provide antenv.axon_hooks (missing in this image) ----------------
_HOOK = [None]
try:
    import antenv.axon_hooks  # noqa: F401
except ImportError:
    try:
        from trn_agent_boot.trn_boot import _ntff_profile_via_ctypes

        _HOOK[0] = _ntff_profile_via_ctypes("/opt/axon/libaxon_pjrt.so")
    except Exception:
        pass
    _m = types.ModuleType("antenv.axon_hooks")
    _m.get_axon_ntff_profile_hook = lambda: _HOOK[0]
    _m.set_axon_ntff_profile_hook = lambda h: _HOOK.__setitem__(0, h)
    sys.modules["antenv.axon_hooks"] = _m
# -----------------------------------------------------------------------------

import concourse.bass as bass
import concourse.mybir as mybir
import concourse.tile as tile
from concourse.bass_utils import run_bass_kernel_spmd
from concourse.vector_clock import ScopedClock


# ---- walrus-compat patches: this walrus rejects Drains with >1 sem wait -----
def _my_drain_and_barrier(self, tick_clock, wait_clock):
    nc = self.nc
    dummy = nc.sync.nop(nofuse=True)
    wait_clock.add_sem_waits(dummy.ins, ScopedClock({None: tick_clock.global_clock}))
    si = dummy.ins.sync_info
    waits = list(si.on_wait) if si is not None else []
    if si is not None and len(waits) > 1:
        dummy.ins.sync_info = mybir.SyncInfo(
            on_wait=[waits[0]], on_update=list(si.on_update)
        )
        for w in waits[1:]:
            n = nc.sync.nop(nofuse=True)
            n.ins.sync_info = mybir.SyncInfo(on_wait=[w], on_update=[])
    nc.sync.drain()
    nc.all_engine_barrier()
    assert self.sems is not None
    popped = nc._tile_sem_poison_stack.pop()
    assert popped is self._sem_poison
    nc.clear_and_free_semaphores(list(self.sems.allocated().values()))
    nc.all_engine_barrier()


def _my_multi_engine_barrier(self, engines):
    for e in engines:
        self.engines[e].drain()
    for inst in self._sem_only_all_engine_barrier_insts(f"aeb{self.next_id()}"):
        self.engines[inst.engine].add_instruction(inst)


tile.TileContext._drain_and_barrier = _my_drain_and_barrier
bass.Bass.multi_engine_barrier = _my_multi_engine_barrier


def _split_multiwait(nc):
    """This walrus build encodes at most one sync-wait per instruction; hoist
    extra waits onto preceding nofuse NOPs on the same engine."""
    n_new = 0
    for fn in nc.m.functions:
        for blk in fn.blocks:
            insts = blk.instructions
            i = 0
            while i < len(insts):
                ins = insts[i]
                si = getattr(ins, "sync_info", None)
                if si is not None and si.on_wait and len(si.on_wait) > 1:
                    waits = list(si.on_wait)
                    ins.sync_info = mybir.SyncInfo(
                        on_wait=[waits[-1]], on_update=list(si.on_update)
                    )
                    new_nops = []
                    for w in waits[:-1]:
                        nop = mybir.InstNoOp(
                            name=f"{ins.name}_wsplit{n_new}",
                            engine=ins.engine,
                            sync_info=mybir.SyncInfo(on_wait=[w], on_update=[]),
                            bass_nofuse=True,
                        )
                        n_new += 1
                        new_nops.append(nop)
                    insts[i:i] = new_nops
                    i += len(new_nops)
                i += 1
    return nc
# -----------------------------------------------------------------------------

T, B, V, L = 512, 32, 8000, 100
NCORES = 8
NB = B // NCORES  # 4 samples per core
WF = 4 * NB  # 16: fused op width (blankF blankB | labelF labelB)
WA = WF + 2 * NB  # 24: alpha tile width (8 zero + 8 blank + 8 label)
NBLK = L + 1  # 101 blank states / state-slots
NLAB = L  # 100 label states
TM = T // 2  # 256 emissions per chain
NCHUNK = 8  # pg DMA / exp chunks
F32 = mybir.dt.float32
F16 = mybir.dt.float16
BF16 = mybir.dt.bfloat16
F8 = mybir.dt.float8e4

# host-side normalization: c_t = log(mean_s exp(e_t(s))) + CNORM
CNORM = np.log(2.0) + 0.25


def build_shift_weight():
    """lhsT [NLAB, NBLK] with w[k, k+1] = 1: ps[j] = label[j-1]."""
    w = np.zeros((NLAB, NBLK), np.float32)
    for k in range(NLAB):
        w[k, k + 1] = 1.0
    return w


def build_program(t_steps=T):
    nc = bass.Bass("TRN2", target_bir_lowering=False, debug=False)
    tm = t_steps // 2
    SROWS = 128
    ntile = NB * t_steps // SROWS
    ABUFS = 4

    acts_d = nc.dram_tensor("acts", [NB * t_steps, V], F32, kind="ExternalInput")
    # chunk-major, rows padded to 112 per chunk: the DMA descriptor
    # distributor spreads rows across engines only when the row count divides
    # evenly (101 is prime -> one engine; 112 = 16x7 -> all 16 engines)
    csz = tm * WF // NCHUNK
    NPAD = 112
    pg_d = nc.dram_tensor("pg", [NCHUNK * NPAD, csz], F16, kind="ExternalInput")
    w_n0_d = nc.dram_tensor("w_n0", [NLAB, NBLK], BF16, kind="ExternalInput")
    e0mask_d = nc.dram_tensor("e0mask", [NBLK, WF], BF16, kind="ExternalInput")

    zout_d = nc.dram_tensor("zout", [SROWS, ntile + 1], F32, kind="ExternalOutput")
    afin_d = nc.dram_tensor("afin", [NBLK, WF], F32, kind="ExternalOutput")

    with tile.TileContext(nc) as tc:
        with (
            tc.tile_pool(name="stream", bufs=2) as stream_pool,
            tc.tile_pool(name="escratch", bufs=1) as escratch_pool,
            tc.tile_pool(name="singles", bufs=1) as singles,
            tc.tile_pool(name="alf", bufs=ABUFS) as al_pool,
            tc.tile_pool(name="qy", bufs=3) as qy_pool,
            tc.tile_pool(name="psp", bufs=4, space="PSUM") as ps_pool,
        ):
            # ---- static small inputs -> SBUF (SP queue) ---------------------
            w_n0 = singles.tile([NLAB, NBLK], BF16)
            e0mask = singles.tile([NBLK, WF], BF16)
            nc.sync.dma_start(out=w_n0, in_=w_n0_d[:, :])
            nc.sync.dma_start(out=e0mask, in_=e0mask_d[:, :])

            # ---- pg chunks on the gpsimd queue: they slip through before the
            #      acts stream fills the engine FIFOs ------------------------
            pg_tiles = []
            for i in range(NCHUNK):
                pgc = singles.tile([NPAD, csz], F16, tag=f"pg{i}")
                nc.gpsimd.dma_start(out=pgc, in_=pg_d[i * NPAD : (i + 1) * NPAD, :])
                pg_tiles.append(pgc)

            # ---- emission table: phat = exp(pg), bf16, chunked --------------
            phat = singles.tile([NBLK, tm * WF], BF16)
            for i in range(NCHUNK):
                sl = slice(i * csz, (i + 1) * csz)
                nc.scalar.activation(
                    phat[:, sl], pg_tiles[i][0:NBLK, :],
                    mybir.ActivationFunctionType.Exp,
                )

            # ---- pre-zero alpha buffers (zero cols persist across reuse) ----
            for i in range(ABUFS):
                a = al_pool.tile([NBLK, WA], BF16, tag="al")
                nc.vector.memset(a, 0.0)

            # ---- init: alpha0 = phat_0 * e0mask (states 0 and 1 only) -------
            alpha = al_pool.tile([NBLK, WA], BF16, tag="al")
            nc.vector.tensor_mul(alpha[:, 2 * NB : WA], phat[:, 0:WF], e0mask)

            # ---- fused twin alpha recursion ---------------------------------
            LABOFF = 4 * NB  # label cols start (16)

            def lab_dup(al):
                base = al[0:NLAB, LABOFF:WA]
                return bass.AP(
                    tensor=al.tensor,
                    offset=base.offset,
                    ap=[list(base.ap[0]), [0, 2], [1, 2 * NB]],
                )

            for t in range(1, tm):
                tsl = slice(t * WF, (t + 1) * WF)
                ps = ps_pool.tile([NBLK, WF], F32, tag="ps")
                nc.tensor.matmul(ps, w_n0, lab_dup(alpha), start=True, stop=True)
                q = qy_pool.tile([NBLK, WF], BF16, tag="q")
                nc.gpsimd.tensor_add(q, alpha[:, 2 * NB : WA], alpha[:, 0 : 4 * NB])
                y = qy_pool.tile([NBLK, WF], BF16, tag="y")
                nc.vector.tensor_add(y, q, ps[0:NBLK, :])
                alpha_n = al_pool.tile([NBLK, WA], BF16, tag="al")
                nc.vector.tensor_mul(alpha_n[:, 2 * NB : WA], y, phat[:, tsl])
                alpha = alpha_n

            # ---- final alphas -> f32 (DMA emitted after stream DMAs) --------
            aff = singles.tile([NBLK, WF], F32)
            nc.vector.tensor_copy(aff, alpha[:, 2 * NB : WA])

            # ---- streaming Z = sum_v exp(acts) ------------------------------
            ztile = singles.tile([SROWS, ntile + 1], F32)
            for it in range(ntile):
                rsl = slice(it * SROWS, (it + 1) * SROWS)
                if it < ntile - 1:
                    tile_a = stream_pool.tile([SROWS, V], F32, tag="acts")
                    nc.sync.dma_start(out=tile_a, in_=acts_d[rsl, :])
                else:
                    # split the last tile column-wise: halves the tail exp
                    tile_a = stream_pool.tile([SROWS, V], F32, tag="acts")
                    nc.sync.dma_start(
                        out=tile_a[:, 0 : V // 2], in_=acts_d[rsl, 0 : V // 2]
                    )
                    nc.sync.dma_start(
                        out=tile_a[:, V // 2 : V], in_=acts_d[rsl, V // 2 : V]
                    )
                    e_h = escratch_pool.tile([SROWS, V], F8, tag="escr")
                    nc.scalar.activation(
                        e_h[:, 0 : V // 2],
                        tile_a[:, 0 : V // 2],
                        mybir.ActivationFunctionType.Exp,
                        accum_out=ztile[:, it : it + 1],
                    )
                    nc.scalar.activation(
                        e_h[:, V // 2 : V],
                        tile_a[:, V // 2 : V],
                        mybir.ActivationFunctionType.Exp,
                        accum_out=ztile[:, it + 1 : it + 2],
                    )
                    break
                e_t = escratch_pool.tile([SROWS, V], F8, tag="escr")
                nc.scalar.activation(
                    e_t,
                    tile_a,
                    mybir.ActivationFunctionType.Exp,
                    accum_out=ztile[:, it : it + 1],
                )

            # outputs on the gpsimd queue: they wait on late producers and
            # must not block the SP stream issue
            nc.gpsimd.dma_start(out=afin_d[:, :], in_=aff)
            nc.gpsimd.dma_start(out=zout_d[:, :], in_=ztile)

    _split_multiwait(nc)
    return nc


_NC_CACHE = {}
_HOST_CACHE = {}


def _get_program(t_steps=T):
    if t_steps not in _NC_CACHE:
        _NC_CACHE[t_steps] = build_program(t_steps)
    return _NC_CACHE[t_steps]


def make_in_maps(acts, targets, t_steps=T):
    """Host prep: gathered+normalized fused emission table (fwd & bwd)."""
    tm = t_steps // 2
    w_n0 = build_shift_weight().astype(ml_dtypes.bfloat16)
    e0mask = np.zeros((NBLK, WF), np.float32)
    e0mask[0, :] = 1.0
    e0mask = e0mask.astype(ml_dtypes.bfloat16)

    # per-(t, sample) normalization constants from gathered acts (float64)
    S = 2 * L + 1
    ext = np.zeros((B, S), np.int64)
    ext[:, 1::2] = targets
    e_all = np.take_along_axis(
        acts.astype(np.float64), np.broadcast_to(ext[None], (t_steps, B, S)), axis=2
    )
    c_all = np.log(np.mean(np.exp(e_all), axis=2)) + CNORM  # [T, B]
    _HOST_CACHE["c_sum"] = c_all.sum(axis=0)  # [B]

    in_maps = []
    for c in range(NCORES):
        bs = slice(c * NB, (c + 1) * NB)
        acts_c = np.ascontiguousarray(
            acts[:t_steps, bs, :].transpose(1, 0, 2).reshape(NB * t_steps, V)
        )
        tg = targets[bs]  # [NB, L]
        a = acts[:t_steps, bs, :]  # [T, NB, V]
        cc = c_all[:, bs]  # [T, NB]

        # fused table cols per t: [blankF(4) blankB(4) labelF(4) labelB(4)]
        pgt = np.full((NBLK, tm, WF), -1e4, np.float32)
        # forward: t = 0..tm-1
        gat = a[:tm, np.arange(NB)[:, None], tg]  # [tm, NB, L]
        pgt[0:NLAB, :, 2 * NB : 3 * NB] = (gat - cc[:tm, :, None]).transpose(2, 0, 1)
        pgt[:, :, 0:NB] = (a[:tm, :, 0] - cc[:tm])[None, :, :]
        # backward: tau = 0..tm-1 maps to t = T-1-tau, reversed label order
        a_r = a[: tm - 1 : -1]  # [tm, NB, V]
        cc_r = cc[: tm - 1 : -1]  # [tm, NB]
        tg_r = tg[:, ::-1]
        gat_r = a_r[:, np.arange(NB)[:, None], tg_r]  # [tm, NB, L]
        pgt[0:NLAB, :, 3 * NB : WF] = (gat_r - cc_r[:, :, None]).transpose(2, 0, 1)
        pgt[:, :, NB : 2 * NB] = (a_r[:, :, 0] - cc_r)[None, :, :]
        ncsz = tm * WF // NCHUNK
        NPAD = 112
        pgc = np.zeros((NCHUNK, NPAD, ncsz), np.float16)
        pgc[:, 0:NBLK, :] = (
            pgt.reshape(NBLK, NCHUNK, ncsz).transpose(1, 0, 2).astype(np.float16)
        )
        pgt = np.ascontiguousarray(pgc.reshape(NCHUNK * NPAD, ncsz))

        in_maps.append(
            {"acts": acts_c, "pg": pgt, "w_n0": w_n0, "e0mask": e0mask}
        )
    return in_maps


def finalize(results, t_steps=T):
    """Host combine: meet-in-the-middle join + normalization + logZ (f64)."""
    S = 2 * L + 1
    ntchunk = t_steps // 128
    c_sum = _HOST_CACHE["c_sum"]
    lls = []
    for c in range(NCORES):
        out = results[c]
        zout = out["zout"].astype(np.float64)  # [128, ntile+1]
        # last tile was exp'd in two column halves: merge the split sums
        zout = np.concatenate(
            [zout[:, :-2], (zout[:, -2] + zout[:, -1])[:, None]], axis=1
        )
        afin = out["afin"].astype(np.float64)  # [NBLK, WF]
        for b in range(NB):
            # flat alpha at t = tm-1 (fwd blocks: cols b and 2NB+b)
            al = np.zeros(S)
            al[0::2] = afin[:, b]
            al[1::2] = afin[0:NLAB, 2 * NB + b]
            # flat beta~ in reversed coords (bwd blocks: cols NB+b, 3NB+b)
            bt = np.zeros(S)
            bt[0::2] = afin[:, NB + b]
            bt[1::2] = afin[0:NLAB, 3 * NB + b]
            beta = bt[::-1]
            # G(s) = beta[s] + beta[s+1] + (s odd)*beta[s+2]
            G = beta.copy()
            G[:-1] += beta[1:]
            G[1:-2:2] += beta[3::2]
            P = float(np.dot(al, G))
            logz = np.log(zout[:, b * ntchunk : (b + 1) * ntchunk]).sum()
            bg = c * NB + b
            lls.append(np.log(P) + c_sum[bg] - logz)
    return -np.sum(lls) / B


def kernel(acts, targets, act_lens, label_lens):
    acts = np.asarray(acts, np.float32)
    targets = np.asarray(targets).astype(np.int64)
    act_lens = np.asarray(act_lens)
    label_lens = np.asarray(label_lens)
    assert acts.shape == (T, B, V), acts.shape
    assert targets.shape == (B, L)
    assert (act_lens == T).all() and (label_lens == L).all(), "only full lens supported"
    assert (targets[:, 1:] != targets[:, :-1]).all(), "adjacent repeats unsupported"

    nc = _get_program(T)
    in_maps = make_in_maps(acts, targets, T)
    res = run_bass_kernel_spmd(nc, in_maps, core_ids=list(range(NCORES)))
    return np.float32(finalize(res.results, T))


if __name__ == "__main__":
    rng = np.random.default_rng(0)
    acts = rng.standard_normal((T, B, V)).astype(np.float32)
    targets = rng.integers(1, V, (B, L)).astype(np.int32)
    for bb in range(B):
        while (targets[bb, 1:] == targets[bb, :-1]).any():
            targets[bb] = rng.integers(1, V, (L,)).astype(np.int32)
    act_lens = np.full(B, T, np.int32)
    label_lens = np.full(B, L, np.int32)
    out = kernel(acts, targets, act_lens, label_lens)
    print("kernel loss:", out)
    from ctc_numpy import ctc_ref_numpy

    ref = ctc_ref_numpy(acts, targets, act_lens, label_lens)
    print("ref    loss:", ref, " rel err:", abs(out - ref) / abs(ref))


# revision 38
# speedup vs baseline: 1.0176x; 1.0176x over previous
"""CTC loss (warp-ctc semantics, size_average=True) on 8 Trainium2 NeuronCores.

Strategy (data-parallel over batch, 4 samples per core):
- Z[t,b] = sum_v exp(acts[t,b,v]) streamed as [128,8000] tiles; exp + free-dim
  sum fused in one ScalarE activation (accum_out) accumulating into an SBUF
  column; one tiny DMA at the end. Host does log Z in float64.
- The alpha recursion runs in the LINEAR domain on p~ = exp(e - c_t), where
  c_t is a host-computed per-(t,sample) normalization constant chosen so the
  recursion stays centered in fp32/bf16 range with NO device rescaling (the
  constants are folded back in log-space on the host, exploiting linearity).
- Forward + backward from both ends, meeting at T/2: the CTC lattice is
  symmetric under time+state reversal (no adjacent repeats), so the backward
  suffix probabilities come from the SAME recursion run on a reversed
  emission table. Both chains are FUSED into shared 16-wide tiles, so each
  time index is ONE bf16 matmul (shift, one resident stationary) plus three
  DVE ops covering both chains:
    q = alpha[:, 8:24] + alpha[:, 0:16]   # [blank|label+blank], zeros trick
    ps = w_shift^T @ dup(label cols)      # TensorE -> PSUM, both chains
    y = q + ps ; alpha' = y * phat_t      # DVE
- Alpha layout [101, 24]: cols 0-7 permanently zero, 8-15 blank (fwd 4, bwd
  4), 16-23 label (fwd 4, bwd 4).
- Final: host combines alpha_mid/beta_mid across the cut in float64 with one
  transition application; ll = log P + sum c_t - sum log Z; loss = -mean(ll).
"""

import sys
import types

import numpy as np
import ml_dtypes

# ---- shim: provide antenv.axon_hooks (missing in this image) ----------------
_HOOK = [None]
try:
    import antenv.axon_hooks  # noqa: F401
except ImportError:
    try:
        from trn_agent_boot.trn_boot import _ntff_profile_via_ctypes

        _HOOK[0] = _ntff_profile_via_ctypes("/opt/axon/libaxon_pjrt.so")
    except Exception:
        pass
    _m = types.ModuleType("antenv.axon_hooks")
    _m.get_axon_ntff_profile_hook = lambda: _HOOK[0]
    _m.set_axon_ntff_profile_hook = lambda h: _HOOK.__setitem__(0, h)
    sys.modules["antenv.axon_hooks"] = _m
# -----------------------------------------------------------------------------

import concourse.bass as bass
import concourse.mybir as mybir
import concourse.tile as tile
from concourse.bass_utils import run_bass_kernel_spmd
from concourse.vector_clock import ScopedClock


# ---- walrus-compat patches: this walrus rejects Drains with >1 sem wait -----
def _my_drain_and_barrier(self, tick_clock, wait_clock):
    nc = self.nc
    dummy = nc.sync.nop(nofuse=True)
    wait_clock.add_sem_waits(dummy.ins, ScopedClock({None: tick_clock.global_clock}))
    si = dummy.ins.sync_info
    waits = list(si.on_wait) if si is not None else []
    if si is not None and len(waits) > 1:
        dummy.ins.sync_info = mybir.SyncInfo(
            on_wait=[waits[0]], on_update=list(si.on_update)
        )
        for w in waits[1:]:
            n = nc.sync.nop(nofuse=True)
            n.ins.sync_info = mybir.SyncInfo(on_wait=[w], on_update=[])
    nc.sync.drain()
    nc.all_engine_barrier()
    assert self.sems is not None
    popped = nc._tile_sem_poison_stack.pop()
    assert popped is self._sem_poison
    nc.clear_and_free_semaphores(list(self.sems.allocated().values()))
    nc.all_engine_barrier()


def _my_multi_engine_barrier(self, engines):
    for e in engines:
        self.engines[e].drain()
    for inst in self._sem_only_all_engine_barrier_insts(f"aeb{self.next_id()}"):
        self.engines[inst.engine].add_instruction(inst)


tile.TileContext._drain_and_barrier = _my_drain_and_barrier
bass.Bass.multi_engine_barrier = _my_multi_engine_barrier


def _split_multiwait(nc):
    """This walrus build encodes at most one sync-wait per instruction; hoist
    extra waits onto preceding nofuse NOPs on the same engine."""
    n_new = 0
    for fn in nc.m.functions:
        for blk in fn.blocks:
            insts = blk.instructions
            i = 0
            while i < len(insts):
                ins = insts[i]
                si = getattr(ins, "sync_info", None)
                if si is not None and si.on_wait and len(si.on_wait) > 1:
                    waits = list(si.on_wait)
                    ins.sync_info = mybir.SyncInfo(
                        on_wait=[waits[-1]], on_update=list(si.on_update)
                    )
                    new_nops = []
                    for w in waits[:-1]:
                        nop = mybir.InstNoOp(
                            name=f"{ins.name}_wsplit{n_new}",
                            engine=ins.engine,
                            sync_info=mybir.SyncInfo(on_wait=[w], on_update=[]),
                            bass_nofuse=True,
                        )
                        n_new += 1
                        new_nops.append(nop)
                    insts[i:i] = new_nops
                    i += len(new_nops)
                i += 1
    return nc
# -----------------------------------------------------------------------------

T, B, V, L = 512, 32, 8000, 100
NCORES = 8
NB = B // NCORES  # 4 samples per core
WF = 4 * NB  # 16: fused op width (blankF blankB | labelF labelB)
WA = WF + 2 * NB  # 24: alpha tile width (8 zero + 8 blank + 8 label)
NBLK = L + 1  # 101 blank states / state-slots
NLAB = L  # 100 label states
TM = T // 2  # 256 emissions per chain
NCHUNK = 8  # pg DMA / exp chunks
F32 = mybir.dt.float32
F16 = mybir.dt.float16
BF16 = mybir.dt.bfloat16
F8 = mybir.dt.float8e4

# host-side normalization: c_t = log(mean_s exp(e_t(s))) + CNORM
CNORM = np.log(2.0) + 0.25


def build_shift_weight():
    """lhsT [NLAB, NBLK] with w[k, k+1] = 1: ps[j] = label[j-1]."""
    w = np.zeros((NLAB, NBLK), np.float32)
    for k in range(NLAB):
        w[k, k + 1] = 1.0
    return w


def build_program(t_steps=T):
    nc = bass.Bass("TRN2", target_bir_lowering=False, debug=False)
    tm = t_steps // 2
    SROWS = 128
    ntile = NB * t_steps // SROWS
    ABUFS = 4

    acts_d = nc.dram_tensor("acts", [NB * t_steps, V], F32, kind="ExternalInput")
    # chunk-major, rows padded to 112 per chunk: the DMA descriptor
    # distributor spreads rows across engines only when the row count divides
    # evenly (101 is prime -> one engine; 112 = 16x7 -> all 16 engines)
    csz = tm * WF // NCHUNK
    NPAD = 112
    pg_d = nc.dram_tensor("pg", [NCHUNK * NPAD, csz], F16, kind="ExternalInput")
    w_n0_d = nc.dram_tensor("w_n0", [NLAB, NBLK], BF16, kind="ExternalInput")
    e0mask_d = nc.dram_tensor("e0mask", [NBLK, WF], BF16, kind="ExternalInput")

    zout_d = nc.dram_tensor("zout", [SROWS, ntile + 1], F32, kind="ExternalOutput")
    afin_d = nc.dram_tensor("afin", [NBLK, WF], F32, kind="ExternalOutput")

    with tile.TileContext(nc) as tc:
        with (
            tc.tile_pool(name="stream", bufs=3) as stream_pool,
            tc.tile_pool(name="escratch", bufs=1) as escratch_pool,
            tc.tile_pool(name="singles", bufs=1) as singles,
            tc.tile_pool(name="alf", bufs=ABUFS) as al_pool,
            tc.tile_pool(name="qy", bufs=3) as qy_pool,
            tc.tile_pool(name="psp", bufs=4, space="PSUM") as ps_pool,
        ):
            # ---- static small inputs -> SBUF (SP queue) ---------------------
            w_n0 = singles.tile([NLAB, NBLK], BF16)
            e0mask = singles.tile([NBLK, WF], BF16)
            nc.sync.dma_start(out=w_n0, in_=w_n0_d[:, :])
            nc.sync.dma_start(out=e0mask, in_=e0mask_d[:, :])

            # ---- pg chunks on the gpsimd queue: they slip through before the
            #      acts stream fills the engine FIFOs ------------------------
            pg_tiles = []
            for i in range(NCHUNK):
                pgc = singles.tile([NPAD, csz], F16, tag=f"pg{i}")
                nc.gpsimd.dma_start(out=pgc, in_=pg_d[i * NPAD : (i + 1) * NPAD, :])
                pg_tiles.append(pgc)

            # ---- emission table: phat = exp(pg), bf16, chunked --------------
            phat = singles.tile([NBLK, tm * WF], BF16)
            for i in range(NCHUNK):
                sl = slice(i * csz, (i + 1) * csz)
                nc.scalar.activation(
                    phat[:, sl], pg_tiles[i][0:NBLK, :],
                    mybir.ActivationFunctionType.Exp,
                )

            # ---- pre-zero alpha buffers (zero cols persist across reuse) ----
            for i in range(ABUFS):
                a = al_pool.tile([NBLK, WA], BF16, tag="al")
                nc.vector.memset(a, 0.0)

            # ---- init: alpha0 = phat_0 * e0mask (states 0 and 1 only) -------
            alpha = al_pool.tile([NBLK, WA], BF16, tag="al")
            nc.vector.tensor_mul(alpha[:, 2 * NB : WA], phat[:, 0:WF], e0mask)

            # ---- fused twin alpha recursion ---------------------------------
            LABOFF = 4 * NB  # label cols start (16)

            def lab_dup(al):
                base = al[0:NLAB, LABOFF:WA]
                return bass.AP(
                    tensor=al.tensor,
                    offset=base.offset,
                    ap=[list(base.ap[0]), [0, 2], [1, 2 * NB]],
                )

            for t in range(1, tm):
                tsl = slice(t * WF, (t + 1) * WF)
                ps = ps_pool.tile([NBLK, WF], F32, tag="ps")
                nc.tensor.matmul(ps, w_n0, lab_dup(alpha), start=True, stop=True)
                q = qy_pool.tile([NBLK, WF], BF16, tag="q")
                nc.gpsimd.tensor_add(q, alpha[:, 2 * NB : WA], alpha[:, 0 : 4 * NB])
                y = qy_pool.tile([NBLK, WF], BF16, tag="y")
                nc.vector.tensor_add(y, q, ps[0:NBLK, :])
                alpha_n = al_pool.tile([NBLK, WA], BF16, tag="al")
                nc.vector.tensor_mul(alpha_n[:, 2 * NB : WA], y, phat[:, tsl])
                alpha = alpha_n

            # ---- final alphas -> f32 (DMA emitted after stream DMAs) --------
            aff = singles.tile([NBLK, WF], F32)
            nc.vector.tensor_copy(aff, alpha[:, 2 * NB : WA])

            # ---- streaming Z = sum_v exp(acts) ------------------------------
            ztile = singles.tile([SROWS, ntile + 1], F32)
            for it in range(ntile):
                rsl = slice(it * SROWS, (it + 1) * SROWS)
                if it < ntile - 1:
                    tile_a = stream_pool.tile([SROWS, V], F32, tag="acts")
                    nc.sync.dma_start(out=tile_a, in_=acts_d[rsl, :])
                else:
                    # split the last tile column-wise: halves the tail exp
                    tile_a = stream_pool.tile([SROWS, V], F32, tag="acts")
                    nc.sync.dma_start(
                        out=tile_a[:, 0 : V // 2], in_=acts_d[rsl, 0 : V // 2]
                    )
                    nc.sync.dma_start(
                        out=tile_a[:, V // 2 : V], in_=acts_d[rsl, V // 2 : V]
                    )
                    e_h = escratch_pool.tile([SROWS, V], F8, tag="escr")
                    nc.scalar.activation(
                        e_h[:, 0 : V // 2],
                        tile_a[:, 0 : V // 2],
                        mybir.ActivationFunctionType.Exp,
                        accum_out=ztile[:, it : it + 1],
                    )
                    nc.scalar.activation(
                        e_h[:, V // 2 : V],
                        tile_a[:, V // 2 : V],
                        mybir.ActivationFunctionType.Exp,
                        accum_out=ztile[:, it + 1 : it + 2],
                    )
                    break
                e_t = escratch_pool.tile([SROWS, V], F8, tag="escr")
                nc.scalar.activation(
                    e_t,
                    tile_a,
                    mybir.ActivationFunctionType.Exp,
                    accum_out=ztile[:, it : it + 1],
                )

            # outputs on the gpsimd queue: they wait on late producers and
            # must not block the SP stream issue
            nc.gpsimd.dma_start(out=afin_d[:, :], in_=aff)
            nc.gpsimd.dma_start(out=zout_d[:, :], in_=ztile)

    _split_multiwait(nc)
    return nc


_NC_CACHE = {}
_HOST_CACHE = {}


def _get_program(t_steps=T):
    if t_steps not in _NC_CACHE:
        _NC_CACHE[t_steps] = build_program(t_steps)
    return _NC_CACHE[t_steps]


def make_in_maps(acts, targets, t_steps=T):
    """Host prep: gathered+normalized fused emission table (fwd & bwd)."""
    tm = t_steps // 2
    w_n0 = build_shift_weight().astype(ml_dtypes.bfloat16)
    e0mask = np.zeros((NBLK, WF), np.float32)
    e0mask[0, :] = 1.0
    e0mask = e0mask.astype(ml_dtypes.bfloat16)

    # per-(t, sample) normalization constants from gathered acts (float64)
    S = 2 * L + 1
    ext = np.zeros((B, S), np.int64)
    ext[:, 1::2] = targets
    e_all = np.take_along_axis(
        acts.astype(np.float64), np.broadcast_to(ext[None], (t_steps, B, S)), axis=2
    )
    c_all = np.log(np.mean(np.exp(e_all), axis=2)) + CNORM  # [T, B]
    _HOST_CACHE["c_sum"] = c_all.sum(axis=0)  # [B]

    in_maps = []
    for c in range(NCORES):
        bs = slice(c * NB, (c + 1) * NB)
        acts_c = np.ascontiguousarray(
            acts[:t_steps, bs, :].transpose(1, 0, 2).reshape(NB * t_steps, V)
        )
        tg = targets[bs]  # [NB, L]
        a = acts[:t_steps, bs, :]  # [T, NB, V]
        cc = c_all[:, bs]  # [T, NB]

        # fused table cols per t: [blankF(4) blankB(4) labelF(4) labelB(4)]
        pgt = np.full((NBLK, tm, WF), -1e4, np.float32)
        # forward: t = 0..tm-1
        gat = a[:tm, np.arange(NB)[:, None], tg]  # [tm, NB, L]
        pgt[0:NLAB, :, 2 * NB : 3 * NB] = (gat - cc[:tm, :, None]).transpose(2, 0, 1)
        pgt[:, :, 0:NB] = (a[:tm, :, 0] - cc[:tm])[None, :, :]
        # backward: tau = 0..tm-1 maps to t = T-1-tau, reversed label order
        a_r = a[: tm - 1 : -1]  # [tm, NB, V]
        cc_r = cc[: tm - 1 : -1]  # [tm, NB]
        tg_r = tg[:, ::-1]
        gat_r = a_r[:, np.arange(NB)[:, None], tg_r]  # [tm, NB, L]
        pgt[0:NLAB, :, 3 * NB : WF] = (gat_r - cc_r[:, :, None]).transpose(2, 0, 1)
        pgt[:, :, NB : 2 * NB] = (a_r[:, :, 0] - cc_r)[None, :, :]
        ncsz = tm * WF // NCHUNK
        NPAD = 112
        pgc = np.zeros((NCHUNK, NPAD, ncsz), np.float16)
        pgc[:, 0:NBLK, :] = (
            pgt.reshape(NBLK, NCHUNK, ncsz).transpose(1, 0, 2).astype(np.float16)
        )
        pgt = np.ascontiguousarray(pgc.reshape(NCHUNK * NPAD, ncsz))

        in_maps.append(
            {"acts": acts_c, "pg": pgt, "w_n0": w_n0, "e0mask": e0mask}
        )
    return in_maps


def finalize(results, t_steps=T):
    """Host combine: meet-in-the-middle join + normalization + logZ (f64)."""
    S = 2 * L + 1
    ntchunk = t_steps // 128
    c_sum = _HOST_CACHE["c_sum"]
    lls = []
    for c in range(NCORES):
        out = results[c]
        zout = out["zout"].astype(np.float64)  # [128, ntile+1]
        # last tile was exp'd in two column halves: merge the split sums
        zout = np.concatenate(
            [zout[:, :-2], (zout[:, -2] + zout[:, -1])[:, None]], axis=1
        )
        afin = out["afin"].astype(np.float64)  # [NBLK, WF]
        for b in range(NB):
            # flat alpha at t = tm-1 (fwd blocks: cols b and 2NB+b)
            al = np.zeros(S)
            al[0::2] = afin[:, b]
            al[1::2] = afin[0:NLAB, 2 * NB + b]
            # flat beta~ in reversed coords (bwd blocks: cols NB+b, 3NB+b)
            bt = np.zeros(S)
            bt[0::2] = afin[:, NB + b]
            bt[1::2] = afin[0:NLAB, 3 * NB + b]
            beta = bt[::-1]
            # G(s) = beta[s] + beta[s+1] + (s odd)*beta[s+2]
            G = beta.copy()
            G[:-1] += beta[1:]
            G[1:-2:2] += beta[3::2]
            P = float(np.dot(al, G))
            logz = np.log(zout[:, b * ntchunk : (b + 1) * ntchunk]).sum()
            bg = c * NB + b
            lls.append(np.log(P) + c_sum[bg] - logz)
    return -np.sum(lls) / B


def kernel(acts, targets, act_lens, label_lens):
    acts = np.asarray(acts, np.float32)
    targets = np.asarray(targets).astype(np.int64)
    act_lens = np.asarray(act_lens)
    label_lens = np.asarray(label_lens)
    assert acts.shape == (T, B, V), acts.shape
    assert targets.shape == (B, L)
    assert (act_lens == T).all() and (label_lens == L).all(), "only full lens supported"
    assert (targets[:, 1:] != targets[:, :-1]).all(), "adjacent repeats unsupported"

    nc = _get_program(T)
    in_maps = make_in_maps(acts, targets, T)
    res = run_bass_kernel_spmd(nc, in_maps, core_ids=list(range(NCORES)))
    return np.float32(finalize(res.results, T))


if __name__ == "__main__":
    rng = np.random.default_rng(0)
    acts = rng.standard_normal((T, B, V)).astype(np.float32)
    targets = rng.integers(1, V, (B, L)).astype(np.int32)
    for bb in range(B):
        while (targets[bb, 1:] == targets[bb, :-1]).any():
            targets[bb] = rng.integers(1, V, (L,)).astype(np.int32)
    act_lens = np.full(B, T, np.int32)
    label_lens = np.full(B, L, np.int32)
    out = kernel(acts, targets, act_lens, label_lens)
    print("kernel loss:", out)
    from ctc_numpy import ctc_ref_numpy

    ref = ctc_ref_numpy(acts, targets, act_lens, label_lens)
    print("ref    loss:", ref, " rel err:", abs(out - ref) / abs(ref))
